# revision 54
# baseline (speedup 1.0000x reference)
"""Trainium2 Bass kernel for nn_AffNet (affinity network).

Reference computation:
    X_emb = X @ W                               # [N, E]
    aff_h = (Z_h @ X_emb^T) / (|X_emb| |Z_h|)   # cosine, [H, N, N]
    aff   = max_h aff_h                          # [N, N]
    aff   = (aff + aff^T) / 2                    # symmetrize
    aff   = (aff + 1) / 2                        # [0, 1]
    aff   = aff ** beta

Device strategy (8 NeuronCores, symmetric block-pair parallel):
  The output is symmetric by construction, so the 16x16 grid of 512x512
  blocks has 120 off-diagonal pairs {(i,j),(j,i)} + 16 diagonal blocks.
  Each core gets 15 pairs + 2 diagonal blocks (exactly 1/8 of the work).
  For a pair, the core computes the pooled block once:
      direct[m, n] = maxP'[m, n] + maxQ'[m, n] + 0.5
  where P'_h = Z''_h[rows_i] . X'[cols_j] and Q'_h = X'[rows_i] . Z''_h[cols_j]
  (normalized operands; x1/4 folded into Z''), which equals
  ((maxP + maxP^T)/2 + 1)/2 on that block, and gets the mirror block
  (j, i) as a TensorE transpose of the direct block — no recompute.
  Diagonal blocks are symmetric by construction and need no mirror.

  SPMD: all cores run the identical program over 17 fixed "slots"; the
  host permutes input columns per core (row-block / col-block copies)
  and scatters the 32 output blocks into the final matrix (adding the
  final +0.5 and upcasting the bf16 device output to fp32 there).

  Per [128, 512] output tile: 8 matmuls (4 P heads + 4 Q heads) into
  eight PSUM banks (heads 1,3 through two single-bank ScalarE-evacuated
  chains; heads 2,4 into two 2-bank tiles consumed by VectorE); ScalarE
  evacuates 4 blocks to bf16, VectorE does two fused L1 maxes (fp32 PSUM
  x bf16 SBUF), one strided bf16 2x L2 max, and a bf16 2x final add.
  Mirror blocks flow through two dedicated transpose PSUM banks.
  Engine balance (cost model, per core): DVE ~214us (bottleneck, 92%
  busy), ScalarE ~193us, PE ~125us, DMA ~109us -> ~232us total.
"""

import numpy as np

N_NODES = 8192
N_FEATURES = 512
EMB = 128
N_HEADS = 4
EPS = 1e-6
N_CORES = 8
BLK = 512                     # symmetric block size
N_BLK = N_NODES // BLK        # 16 row/col blocks
M_CHUNK = 128                 # rows per matmul (PSUM partitions)
N_PAIRS = 15                  # off-diagonal pairs per core
N_DIAG = 2                    # diagonal blocks per core
N_SLOTS = N_PAIRS + N_DIAG    # 17
SLOT_COLS = N_SLOTS * BLK     # 8704

_CACHE = {}
LAST_RESULT = None


def _assignments():
    """Global block->core assignment, identical on every call."""
    pairs = [(i, j) for i in range(N_BLK) for j in range(i + 1, N_BLK)]
    diags = [(i, i) for i in range(N_BLK)]
    per_core = []
    for c in range(N_CORES):
        my = pairs[c::N_CORES] + diags[c::N_CORES]
        assert len(my) == N_SLOTS
        per_core.append(my)
    return per_core


def _split_multi_waits(nc, limit=1):
    """The walrus build in this environment encodes at most one semaphore
    wait per instruction ("Too many sync wait commands" otherwise), while
    Tile attaches several. Hoist extra waits onto same-engine NOPs inserted
    immediately before the instruction (waits still execute before it)."""
    import concourse.mybir as mybir

    for f in nc.m.functions:
        for bb in f.blocks:
            il = bb.instructions  # live list backing the block
            idx = 0
            while idx < len(il):
                inst = il[idx]
                si = inst.sync_info
                waits = list(si.on_wait) if si is not None and si.on_wait else []
                if len(waits) > limit:
                    ups = list(si.on_update) if si.on_update else []
                    inst.sync_info = mybir.SyncInfo(
                        on_wait=waits[:limit], on_update=ups
                    )
                    eng = nc.engines[inst.engine]
                    pos = idx
                    for j in range(limit, len(waits), limit):
                        nbi = eng.nop()
                        ninst = nbi.ins
                        # nop() appended itself to the current bb; detach it
                        removed = False
                        for f2 in nc.m.functions:
                            for bb2 in f2.blocks:
                                l2 = bb2.instructions
                                if l2 and l2[-1].name == ninst.name:
                                    l2.pop()
                                    removed = True
                                    break
                            if removed:
                                break
                        assert removed, "could not detach helper nop"
                        ninst.sync_info = mybir.SyncInfo(
                            on_wait=waits[j : j + limit], on_update=[]
                        )
                        il.insert(pos, ninst)
                        pos += 1
                        idx += 1
                idx += 1


def _build_program():
    import concourse.bass as bass
    import concourse.mybir as mybir
    import concourse.tile as tile
    from concourse.masks import make_identity

    nc = bass.Bass("TRN2", target_bir_lowering=False, debug=False)

    bf16 = mybir.dt.bfloat16
    f32 = mybir.dt.float32
    # Per-core slot-major operands (host packs [slot][E, BLK] slices)
    xr = nc.dram_tensor("xr", [N_PAIRS, EMB, BLK], bf16, kind="ExternalInput")
    xc = nc.dram_tensor("xc", [N_SLOTS, EMB, BLK], bf16, kind="ExternalInput")
    zr = nc.dram_tensor("zr", [N_HEADS, N_SLOTS, EMB, BLK], bf16,
                        kind="ExternalInput")
    zc = nc.dram_tensor("zc", [N_HEADS, N_PAIRS, EMB, BLK], bf16,
                        kind="ExternalInput")
    # direct blocks [17, 512, 512] + mirror blocks [15, 512, 512]
    # bf16 outputs: host upcasts and adds the final +0.5 during assembly
    outd = nc.dram_tensor("outd", [N_SLOTS, BLK, BLK], bf16, kind="ExternalOutput")
    outm = nc.dram_tensor("outm", [N_PAIRS, BLK, BLK], bf16, kind="ExternalOutput")

    n_m = BLK // M_CHUNK  # 4 m-chunks per block

    with tile.TileContext(nc) as tc:
        with (
            tc.tile_pool(name="weights", bufs=1) as wpool,
            tc.tile_pool(name="psum", bufs=1, space="PSUM") as ppool,
            tc.tile_pool(name="work", bufs=2) as spool,
        ):
            ident = wpool.tile([128, 128], bf16, tag="ident")
            make_identity(nc, ident)

            for s in range(N_SLOTS):
                is_diag = s >= N_PAIRS
                # per-slot input tiles (multi-buffered so prefetch
                # overlaps); xc + zr first: the first matmuls need them.
                # Diagonal slots never touch xr/zc, so skip those loads.
                xc_s = spool.tile([EMB, BLK], bf16, tag="xc", bufs=4,
                                  name=f"xc_{s}")
                nc.sync.dma_start(out=xc_s, in_=xc[s])
                zr_s, zc_s = [], []
                for h in range(N_HEADS):
                    t = spool.tile([EMB, BLK], bf16, tag=f"zr{h}", bufs=4,
                                   name=f"zr{h}_{s}")
                    nc.sync.dma_start(out=t, in_=zr[h, s])
                    zr_s.append(t)
                if not is_diag:
                    xr_s = spool.tile([EMB, BLK], bf16, tag="xr", bufs=4,
                                      name=f"xr_{s}")
                    nc.sync.dma_start(out=xr_s, in_=xr[s])
                    for h in range(N_HEADS):
                        t = spool.tile([EMB, BLK], bf16, tag=f"zc{h}", bufs=4,
                                       name=f"zc{h}_{s}")
                        nc.sync.dma_start(out=t, in_=zc[h, s])
                        zc_s.append(t)

                if is_diag:
                    # Diagonal block: P[A,A] and Q[A,A] are transposes of
                    # each other, so compute only the P matmuls, pool the 4
                    # heads, and finish with out = maxP + maxP^T via
                    # TensorE transposes. Half the matmuls and pooling.
                    dmx = []  # pooled maxP tiles, [128, 4, 128] bf16
                    for m in range(n_m):
                        msl = slice(m * M_CHUNK, (m + 1) * M_CHUNK)
                        ap1 = ppool.tile([M_CHUNK, BLK], f32, tag="ap_a",
                                         name=f"dap1_{s}_{m}")
                        b1 = ppool.tile([M_CHUNK, 2, BLK], f32, tag="b1",
                                        name=f"db1_{s}_{m}")
                        nc.tensor.matmul(ap1, zr_s[0][:, msl], xc_s,
                                         start=True, stop=True)
                        nc.tensor.matmul(b1[:, 0], zr_s[1][:, msl], xc_s,
                                         start=True, stop=True)
                        ea = spool.tile([M_CHUNK, 4, BLK], bf16, tag="ea",
                                        bufs=4, name=f"dea_{s}_{m}")
                        nc.scalar.copy(ea[:, 0], ap1)
                        ap3 = ppool.tile([M_CHUNK, BLK], f32, tag="ap_b",
                                         name=f"dap3_{s}_{m}")
                        nc.tensor.matmul(b1[:, 1], zr_s[3][:, msl], xc_s,
                                         start=True, stop=True)
                        nc.tensor.matmul(ap3, zr_s[2][:, msl], xc_s,
                                         start=True, stop=True)
                        nc.scalar.copy(ea[:, 1], ap3)
                        l1 = spool.tile([M_CHUNK, 2, BLK], bf16, tag="l1",
                                        bufs=3, name=f"dl1_{s}_{m}")
                        nc.vector.tensor_max(l1, b1, ea[:, 0:2])
                        dm = spool.tile([M_CHUNK, 4, M_CHUNK], bf16,
                                        tag="dmx", bufs=5, name=f"dmx_{s}_{m}")
                        nc.vector.tensor_max(dm, l1[:, 0], l1[:, 1])
                        dmx.append(dm)
                    for mp2 in range(n_m // 2):
                        tp = ppool.tile([M_CHUNK, 8, M_CHUNK], bf16,
                                        tag="tp", name=f"dtp_{s}_{mp2}")
                        for half in range(2):
                            mp = 2 * mp2 + half
                            for m in range(n_m):
                                nc.tensor.transpose(tp[:, 4 * half + m],
                                                    dmx[m][:, mp], ident)
                        for half in range(2):
                            mp = 2 * mp2 + half
                            o = spool.tile([M_CHUNK, 4, M_CHUNK], bf16,
                                           tag="o", bufs=4,
                                           name=f"do_{s}_{mp}")
                            nc.vector.tensor_add(
                                o, dmx[mp], tp[:, 4 * half:4 * half + 4])
                            nc.gpsimd.dma_start(
                                out=outd[s, mp * M_CHUNK:(mp + 1) * M_CHUNK, :],
                                in_=o,
                            )
                    continue

                direct = []  # views of the 4 [128, 512] bf16 out tiles
                l1d = od = None
                for m in range(n_m):
                    msl = slice(m * M_CHUNK, (m + 1) * M_CHUNK)
                    half4 = 4 * (m % 2)
                    # PSUM bank map (8 banks):
                    #   ap_a {P1}, ap_b {P3}, aq {Q1 then Q3} -- ScalarE
                    #     evacuates these fast (short independent chains).
                    #   b1 {P2,Q2}, b2 {P4,Q4} -- freed by the two fused
                    #     VectorE L1 maxes (the pipeline bottleneck).
                    #   tp -- dedicated transpose bank, keeping the mirror
                    #     path off the matmul critical path.
                    ap1 = ppool.tile([M_CHUNK, BLK], f32, tag="ap_a",
                                     name=f"ap1_{s}_{m}")
                    aq1 = ppool.tile([M_CHUNK, BLK], f32, tag="aq",
                                     name=f"aq1_{s}_{m}")
                    b1 = ppool.tile([M_CHUNK, 2, BLK], f32, tag="b1",
                                    name=f"b1_{s}_{m}")
                    b2 = ppool.tile([M_CHUNK, 2, BLK], f32, tag="b2",
                                    name=f"b2_{s}_{m}")
                    nc.tensor.matmul(ap1, zr_s[0][:, msl], xc_s,
                                     start=True, stop=True)
                    nc.tensor.matmul(aq1, xr_s[:, msl], zc_s[0],
                                     start=True, stop=True)
                    nc.tensor.matmul(b1[:, 0], zr_s[1][:, msl], xc_s,
                                     start=True, stop=True)
                    nc.tensor.matmul(b1[:, 1], xr_s[:, msl], zc_s[1],
                                     start=True, stop=True)
                    # ScalarE: evacuate A-blocks fp32 -> bf16 SBUF as they
                    # land; two independent single-bank chains (P and Q)
                    # ea layout: {eP1, eP3, eQ1, eQ3}
                    ea = spool.tile([M_CHUNK, 4, BLK], bf16, tag="ea", bufs=4)
                    nc.scalar.copy(ea[:, 0], ap1)
                    nc.scalar.copy(ea[:, 2], aq1)
                    ap3 = ppool.tile([M_CHUNK, BLK], f32, tag="ap_b",
                                     name=f"ap3_{s}_{m}")
                    aq3 = ppool.tile([M_CHUNK, BLK], f32, tag="aq",
                                     name=f"aq3_{s}_{m}")
                    nc.tensor.matmul(b2[:, 0], zr_s[3][:, msl], xc_s,
                                     start=True, stop=True)
                    nc.tensor.matmul(b2[:, 1], xr_s[:, msl], zc_s[3],
                                     start=True, stop=True)
                    nc.tensor.matmul(ap3, zr_s[2][:, msl], xc_s,
                                     start=True, stop=True)
                    nc.tensor.matmul(aq3, xr_s[:, msl], zc_s[2],
                                     start=True, stop=True)
                    nc.scalar.copy(ea[:, 1], ap3)
                    nc.scalar.copy(ea[:, 3], aq3)
                    # VectorE L1: l1 = {m12P, m34P, m12Q, m34Q} per tile,
                    # two tiles sharing one l1 tile so the SBUF-side
                    # combines run once per tile pair at full width
                    if m % 2 == 0:
                        l1d = spool.tile([M_CHUNK, 8, BLK], bf16, tag="l1",
                                         bufs=3, name=f"l1_{s}_{m}")
                    nc.vector.tensor_max(l1d[:, half4 + 0:half4 + 4:2],
                                         b1, ea[:, 0:4:2])
                    nc.vector.tensor_max(l1d[:, half4 + 1:half4 + 4:2],
                                         b2, ea[:, 1:4:2])
                    if m % 2 == 1:
                        # L2 (both tiles): {maxP0, maxQ0, maxP1, maxQ1}
                        l2 = spool.tile([M_CHUNK, 4, BLK], bf16, tag="l2",
                                        bufs=3, name=f"l2_{s}_{m}")
                        nc.vector.tensor_max(l2, l1d[:, 0:8:2], l1d[:, 1:8:2])
                        # final adds (both tiles): out = maxP + maxQ
                        od = spool.tile([M_CHUNK, 2, BLK], bf16, tag="o",
                                        bufs=4, name=f"od_{s}_{m}")
                        nc.vector.tensor_add(od, l2[:, 0:4:2], l2[:, 1:4:2])
                        nc.gpsimd.dma_start(
                            out=outd[s, (m - 1) * M_CHUNK:m * M_CHUNK, :],
                            in_=od[:, 0],
                        )
                        nc.gpsimd.dma_start(
                            out=outd[s, m * M_CHUNK:(m + 1) * M_CHUNK, :],
                            in_=od[:, 1],
                        )
                        direct.append(od[:, 0])
                        direct.append(od[:, 1])
                if not is_diag:
                    # mirror block: transpose the 4 direct tiles chunk-wise.
                    # Two mirror tiles' bf16 chunks fit one PSUM bank, so
                    # eight transposes share one fused ScalarE evacuation.
                    for mp2 in range(n_m // 2):
                        tp = ppool.tile([M_CHUNK, 8, M_CHUNK], bf16,
                                        tag="tp", name=f"tp_{s}_{mp2}")
                        for half in range(2):
                            mp = 2 * mp2 + half
                            for m in range(n_m):
                                nc.tensor.transpose(
                                    tp[:, 4 * half + m],
                                    direct[m][:, mp * M_CHUNK:(mp + 1) * M_CHUNK],
                                    ident,
                                )
                        mo = spool.tile([M_CHUNK, 8, M_CHUNK], bf16, tag="mo",
                                        bufs=4)
                        nc.scalar.copy(mo, tp)
                        for half in range(2):
                            mp = 2 * mp2 + half
                            nc.sync.dma_start(
                                out=outm[s, mp * M_CHUNK:(mp + 1) * M_CHUNK, :],
                                in_=mo[:, 4 * half:4 * half + 4],
                            )

    _split_multi_waits(nc)
    return nc


def kernel(X, W, Z, beta):
    global LAST_RESULT
    import ml_dtypes
    from concourse.bass_utils import run_bass_kernel_spmd

    X = np.asarray(X, dtype=np.float32)
    W = np.asarray(W, dtype=np.float32)
    Z = np.asarray(Z, dtype=np.float32)
    beta_f = float(np.asarray(beta))

    # Host: normalized, transposed, bf16 operands
    X_emb = X @ W                                            # [N, E] fp32
    Xn = np.sqrt(np.sum(X_emb * X_emb, axis=-1))             # [N]
    Zn = np.sqrt(np.sum(Z * Z, axis=-1))                     # [H, N]
    Xp = X_emb / (Xn[:, None] + EPS)                         # [N, E]
    Zp = Z / (Zn[:, :, None] + EPS) * 0.25                   # [H, N, E]
    bf16 = ml_dtypes.bfloat16
    XpT = np.ascontiguousarray(Xp.T).astype(bf16)            # [E, N]
    ZpT = np.ascontiguousarray(Zp.transpose(0, 2, 1)).astype(bf16)  # [H, E, N]

    if "nc" not in _CACHE:
        _CACHE["nc"] = _build_program()
    nc = _CACHE["nc"]

    assign = _assignments()
    in_maps = []
    for c in range(N_CORES):
        blocks = assign[c]
        ridx = np.concatenate(
            [np.arange(i * BLK, (i + 1) * BLK) for (i, j) in blocks]
        )
        cidx = np.concatenate(
            [np.arange(j * BLK, (j + 1) * BLK) for (i, j) in blocks]
        )
        def slotize_x(a):  # [E, 17*BLK] -> [17, E, BLK]
            return np.ascontiguousarray(
                a.reshape(EMB, N_SLOTS, BLK).transpose(1, 0, 2)
            )

        def slotize_z(a):  # [H, E, 17*BLK] -> [H, 17, E, BLK]
            return np.ascontiguousarray(
                a.reshape(N_HEADS, EMB, N_SLOTS, BLK).transpose(0, 2, 1, 3)
            )

        in_maps.append(
            {
                "xr": slotize_x(XpT[:, ridx])[:N_PAIRS],
                "xc": slotize_x(XpT[:, cidx]),
                "zr": slotize_z(ZpT[:, :, ridx]),
                "zc": slotize_z(ZpT[:, :, cidx])[:, :N_PAIRS],
            }
        )

    res = None
    for attempt in range(3):
        try:
            res = run_bass_kernel_spmd(nc, in_maps, list(range(N_CORES)))
            break
        except Exception:
            if attempt == 2:
                raise
    LAST_RESULT = res

    outp = np.empty((N_NODES, N_NODES), dtype=np.float32)
    for c in range(N_CORES):
        blocks = assign[c]
        outd = res.results[c]["outd"]
        outm = res.results[c]["outm"]
        for s, (i, j) in enumerate(blocks):
            # device returns maxP + maxQ in bf16; +0.5 finishes the affine
            np.add(outd[s], np.float32(0.5),
                   out=outp[i * BLK:(i + 1) * BLK, j * BLK:(j + 1) * BLK],
                   dtype=np.float32, casting="unsafe")
            if i != j:
                np.add(outm[s], np.float32(0.5),
                       out=outp[j * BLK:(j + 1) * BLK, i * BLK:(i + 1) * BLK],
                       dtype=np.float32, casting="unsafe")

    if beta_f != 1.0:
        outp = np.power(outp, beta_f, dtype=np.float32)
    return outp
